# revision 37
# baseline (speedup 1.0000x reference)
"""CDiT block kernel for 8 TRN2 NeuronCores.

Sharding: core c handles batch b=c//2, token half h=c%2 (512 of 1024 tokens).
Each core computes the full output for its (b, token-half) slice; K/V are
computed over the full T of the batch (duplicated within the pair), so no
cross-core collectives are needed.

Host folds adaLN modulation (scale/shift) and gates into the projection
weights/biases (they are per-batch constants), pre-transposes weights to
[din, dout] bf16, and pre-transposes x to feature-major [D, T] with the
token axis rolled so each core's own tokens are [0:512].

Device layout: activations feature-major [128, d_tiles, T]; matmuls use
weights as stationary lhsT and activations as moving rhs (feature-major out).
V is produced token-major directly (activation tiles stationary, weight
columns moving) in the stacked per-head [vre|vim] layout, so AV needs no
DMA transposes. Scores are computed transposed (S^T, k on partitions) with
stacked-contraction K/Q ([Kr;Ki] etc.), so exp(S^T) feeds AV directly as
the moving operand; softmax denominators come from ones-matmuls, the 1/z
normalization is broadcast across partitions via a ones-column matmul, and
the V bias is folded in with a rank-1 (b x z) matmul before normalizing.
Softmax has no max subtraction (scores are O(1); a fixed -10 bias guards
overflow).
"""

import os
import sys
import numpy as np

for _p in ("/opt/trn_rl_repo",):
    if _p not in sys.path:
        sys.path.insert(0, _p)

import ml_dtypes

import concourse.bass as bass
import concourse.mybir as mybir
import concourse.tile as tile
from concourse.bass_utils import run_bass_kernel_spmd

B, T, D, H = 4, 1024, 512, 8
DH = D // H
MLP = 4 * D
EPS = 1e-6
P = 128
DT = D // P          # 4 feature tiles
TQ = T // 2          # own tokens per core
NCORES = 8

F32 = mybir.dt.float32
BF16 = mybir.dt.bfloat16
FP8 = mybir.dt.float8e4
AF = mybir.ActivationFunctionType
ALU = mybir.AluOpType
BF = ml_dtypes.bfloat16


# ----------------------------------------------------------------------------
# Host-side prep
# ----------------------------------------------------------------------------

def _feat_major(w_t):
    """[din, dout] -> [128, din//128 * dout] with din = kt*128 + p."""
    din, dout = w_t.shape
    return np.ascontiguousarray(
        w_t.reshape(din // P, P, dout).transpose(1, 0, 2).reshape(P, -1)
    )


def _col(v):
    """[dout] -> [128, dout//128] per-partition bias layout (d = o*128+p)."""
    return np.ascontiguousarray(v.reshape(-1, P).T)


def _silu(x):
    return x / (1.0 + np.exp(-x))


def _prep_core(inputs, b, half):
    """Build the per-core input map (numpy arrays, host precomputation)."""
    f = np.float32
    g = lambda n: np.asarray(inputs[n], dtype=f)

    # adaLN on host (tiny): complex silu -> complex linear -> 6 chunks
    sr, si = _silu(g('c_re')[b]), _silu(g('c_im')[b])
    aWr, aWi = g('ada_Wr'), g('ada_Wi')
    m_re = aWr @ sr - aWi @ si + (g('ada_br') - g('ada_bi'))
    m_im = aWr @ si + aWi @ sr + (g('ada_br') + g('ada_bi'))
    sh_ar, sc_ar, g_ar, sh_mr, sc_mr, g_mr = np.split(m_re, 6)
    sh_ai, sc_ai, g_ai, sh_mi, sc_mi, g_mi = np.split(m_im, 6)

    def fold_mod(Wr, Wi, br, bi, a, bb, shr, shi):
        """Fold complex modulate diag(a+ib)+shift into complex linear."""
        Mr = Wr * a[None, :] - Wi * bb[None, :]
        Mi = Wi * a[None, :] + Wr * bb[None, :]
        bre = (br - bi) + Wr @ shr - Wi @ shi
        bim = (br + bi) + Wi @ shr + Wr @ shi
        return Mr, Mi, bre, bim

    a1, b1 = 1.0 + sc_ar, sc_ai
    a2, b2 = 1.0 + sc_mr, sc_mi

    qMr, qMi, qbre, qbim = fold_mod(g('q_Wr'), g('q_Wi'), g('q_br'), g('q_bi'),
                                    a1, b1, sh_ar, sh_ai)
    kMr, kMi, kbre, kbim = fold_mod(g('k_Wr'), g('k_Wi'), g('k_br'), g('k_bi'),
                                    a1, b1, sh_ar, sh_ai)
    vMr, vMi, vbre, vbim = fold_mod(g('v_Wr'), g('v_Wi'), g('v_br'), g('v_bi'),
                                    a1, b1, sh_ar, sh_ai)
    scale = 1.0 / np.sqrt(np.float32(DH))
    qMr, qMi, qbre, qbim = qMr * scale, qMi * scale, qbre * scale, qbim * scale

    f1Mr, f1Mi, f1bre, f1bim = fold_mod(g('f1_Wr'), g('f1_Wi'),
                                        g('f1_br'), g('f1_bi'),
                                        a2, b2, sh_mr, sh_mi)

    # o-proj with attention gate folded (row scaling by complex g_a)
    oWr, oWi = g('o_Wr'), g('o_Wi')
    oGr = g_ar[:, None] * oWr - g_ai[:, None] * oWi
    oGi = g_ai[:, None] * oWr + g_ar[:, None] * oWi
    obre, obim = g('o_br') - g('o_bi'), g('o_br') + g('o_bi')
    ogbre = g_ar * obre - g_ai * obim
    ogbim = g_ai * obre + g_ar * obim

    # f2 with MLP gate folded
    fWr, fWi = g('f2_Wr'), g('f2_Wi')
    fGr = g_mr[:, None] * fWr - g_mi[:, None] * fWi
    fGi = g_mi[:, None] * fWr + g_mr[:, None] * fWi
    fbre, fbim = g('f2_br') - g('f2_bi'), g('f2_br') + g('f2_bi')
    fgbre = g_mr * fbre - g_mi * fbim
    fgbim = g_mi * fbre + g_mr * fbim

    # KA stacked weights: out rows = per head [Kr_h(64); Ki_h(64)]
    kA = np.empty((D * 2, D), f)   # rows for nr
    kB = np.empty((D * 2, D), f)   # rows for ni
    ka_b = np.empty(D * 2, f)
    for h in range(H):
        r = slice(h * DH, (h + 1) * DH)
        blk = slice(h * P, h * P + DH)
        blk2 = slice(h * P + DH, (h + 1) * P)
        kA[blk], kA[blk2] = kMr[r], kMi[r]
        kB[blk], kB[blk2] = -kMi[r], kMr[r]
        ka_b[blk], ka_b[blk2] = kbre[r], kbim[r]

    # V moving packs for token-major V^T: cols c = h*128 + (re 64 | im 64)
    WVAr = np.empty((D, 2 * D), f)   # moving for nr
    WVAi = np.empty((D, 2 * D), f)   # moving for ni
    for h in range(H):
        r = slice(h * DH, (h + 1) * DH)
        WVAr[:, h * P: h * P + DH] = vMr[r].T
        WVAr[:, h * P + DH: (h + 1) * P] = vMi[r].T
        WVAi[:, h * P: h * P + DH] = -vMi[r].T
        WVAi[:, h * P + DH: (h + 1) * P] = vMr[r].T

    # AV bias row (folded post-normalization via b*z rank-1 matmul):
    # [bvre-bvim; bvre+bvim] per head
    av_b = np.empty(D * 2, f)
    for h in range(H):
        r = slice(h * DH, (h + 1) * DH)
        av_b[h * P: h * P + DH] = vbre[r] - vbim[r]
        av_b[h * P + DH: (h + 1) * P] = vbre[r] + vbim[r]
    av_col = _col(av_b)                            # [128, 8]

    bf = lambda w: _feat_major(w).astype(BF)

    wq = np.concatenate([bf(qMr.T), bf(qMi.T), bf(-qMi.T)], axis=1)
    wka0 = np.concatenate([bf(kA.T[:, 0:512]), bf(kB.T[:, 0:512])], axis=1)
    wka1 = np.concatenate([bf(kA.T[:, 512:1024]), bf(kB.T[:, 512:1024])],
                          axis=1)
    wvaR = bf(WVAr)
    wvaI = bf(WVAi)
    wo = np.concatenate([bf(oGr.T), bf(oGi.T), bf(-oGi.T)], axis=1)
    wf1 = [np.concatenate([bf(f1Mr.T[:, j * 512:(j + 1) * 512]),
                           bf(f1Mi.T[:, j * 512:(j + 1) * 512]),
                           bf(-f1Mi.T[:, j * 512:(j + 1) * 512])], axis=1)
           for j in range(4)]
    wf2 = [np.concatenate([bf(fGr.T[j * 512:(j + 1) * 512]),
                           bf(fGi.T[j * 512:(j + 1) * 512]),
                           bf(-fGi.T[j * 512:(j + 1) * 512])], axis=1)
           for j in range(4)]

    smalls = np.concatenate([
        _col(qbre), _col(qbim), _col(-qbim),           # 0:4, 4:8, 8:12
        _col(ka_b),                                    # 12:20
        _col(ogbre), _col(ogbim),                      # 20:24, 24:28
        _col(f1bre), _col(f1bim),                      # 28:44, 44:60
        _col(fgbre), _col(fgbim),                      # 60:64, 64:68
        np.full((P, 1), EPS, f),                       # 68
        av_col,                                        # 69:77
    ], axis=1)

    roll = lambda a: np.roll(a, -half * TQ, axis=0)
    xTr = np.ascontiguousarray(roll(g('x_re')[b]).T)
    xTi = np.ascontiguousarray(roll(g('x_im')[b]).T)

    im = {'xTr': xTr, 'xTi': xTi, 'wq': wq, 'wka0': wka0, 'wka1': wka1,
          'wvaR': wvaR, 'wvaI': wvaI, 'wo': wo, 'smalls': smalls}
    for j in range(4):
        im[f'wf1_{j}'] = wf1[j]
        im[f'wf2_{j}'] = wf2[j]
    return im


# ----------------------------------------------------------------------------
# Device program
# ----------------------------------------------------------------------------

def build_nc(reps=1):
    nc = bass.Bass()

    xTr_d = nc.declare_dram_parameter("xTr", [D, T], F32, isOutput=False)
    xTi_d = nc.declare_dram_parameter("xTi", [D, T], F32, isOutput=False)
    wq_d = nc.declare_dram_parameter("wq", [P, 6144], BF16, isOutput=False)
    wka0_d = nc.declare_dram_parameter("wka0", [P, 4096], BF16, isOutput=False)
    wka1_d = nc.declare_dram_parameter("wka1", [P, 4096], BF16, isOutput=False)
    wvaR_d = nc.declare_dram_parameter("wvaR", [P, 4096], BF16, isOutput=False)
    wvaI_d = nc.declare_dram_parameter("wvaI", [P, 4096], BF16, isOutput=False)
    wo_d = nc.declare_dram_parameter("wo", [P, 6144], BF16, isOutput=False)
    wf1_d = [nc.declare_dram_parameter(f"wf1_{j}", [P, 6144], BF16,
                                       isOutput=False) for j in range(4)]
    wf2_d = [nc.declare_dram_parameter(f"wf2_{j}", [P, 6144], BF16,
                                       isOutput=False) for j in range(4)]
    smalls_d = nc.declare_dram_parameter("smalls", [P, 77], F32, isOutput=False)
    out_d = nc.declare_dram_parameter("outT", [2, D, TQ], F32, isOutput=True)

    with tile.TileContext(nc) as tc:
        with (
            tc.tile_pool(name="persist", bufs=1) as pp,
            tc.tile_pool(name="acts", bufs=1) as ap_,
            tc.tile_pool(name="tmp", bufs=2) as tp,
            tc.tile_pool(name="attn", bufs=2) as atp,
            tc.tile_pool(name="psum", bufs=2, space="PSUM") as psp,
            tc.tile_pool(name="psum_av", bufs=2, space="PSUM") as psa,
            tc.tile_pool(name="psum_st", bufs=1, space="PSUM") as pst,
        ):
            def emit():
                dma = nc.sync.dma_start

                smalls = pp.tile([P, 77], F32)
                dma(smalls, smalls_d.ap())
                b_qre, b_qim, b_nqim = smalls[:, 0:4], smalls[:, 4:8], smalls[:, 8:12]
                b_ka = smalls[:, 12:20]
                b_ore, b_oim = smalls[:, 20:24], smalls[:, 24:28]
                b_f1re, b_f1im = smalls[:, 28:44], smalls[:, 44:60]
                b_f2re, b_f2im = smalls[:, 60:64], smalls[:, 64:68]
                eps = smalls[:, 68:69]
                b_av = smalls[:, 69:77]

                ones = pp.tile([P, 1], BF16)
                nc.vector.memset(ones, 1.0)
                onesw = pp.tile([P, P], BF16)
                nc.vector.memset(onesw, 1.0)
                oneD = pp.tile([P, 1], BF16)
                nc.vector.memset(oneD, 1.0 / D)
                onesb = pp.tile([1, P], BF16)
                nc.vector.memset(onesb, 1.0)
                m10 = pp.tile([P, 1], F32)
                nc.vector.memset(m10, -10.0)

                def loadpack(src, n, q=0):
                    wpk = pp.tile([P, 6144], BF16, tag="wpk", bufs=2, name=n)
                    eng = nc.sync if q == 0 else nc.gpsimd
                    eng.dma_start(wpk[:, 0:src.shape[1]], src.ap())
                    return wpk

                # ---------------- LayerNorm helper (per 512-token chunk) --------
                def ln_chunk(fetch, nout, tag):
                    """fetch(d) -> [P, 512] f32 AP; nout [P, DT, 512] bf16."""
                    xsq = tp.tile([P, DT, 2, 512], BF16, tag="xsq", bufs=2)
                    for d in range(DT):
                        xf = fetch(d)
                        nc.scalar.activation(out=xsq[:, d, 0, :], in_=xf,
                                             func=AF.Copy)
                        nc.scalar.activation(out=xsq[:, d, 1, :], in_=xf,
                                             func=AF.Square)
                    ps = pst.tile([1, 2, 512], F32, tag="st")
                    for s in range(2):
                        for d in range(DT):
                            nc.tensor.matmul(ps[:, s, :], oneD[:, 0:1],
                                             xsq[:, d, s, :],
                                             start=(d == 0), stop=(d == DT - 1))
                    strow = tp.tile([1, 2, 512], BF16, tag="strow", bufs=1)
                    nc.scalar.activation(out=strow, in_=ps, func=AF.Copy)
                    lnvp = pst.tile([P, 2, 512], F32, tag="lnv")
                    for s in range(2):
                        nc.tensor.matmul(lnvp[:, s, :], onesb, strow[0:1, s, :],
                                         start=True, stop=True)
                    lnv = tp.tile([P, 2, 512], F32, tag="lnv", bufs=1)
                    nc.scalar.activation(out=lnv, in_=lnvp, func=AF.Copy)
                    mu, msq = lnv[:, 0, :], lnv[:, 1, :]
                    sc = tp.tile([P, 512], F32, tag="lnsc")
                    nc.vector.tensor_tensor(out=sc, in0=mu, in1=mu, op=ALU.mult)
                    nc.vector.tensor_tensor(out=msq, in0=msq, in1=sc,
                                            op=ALU.subtract)
                    nc.scalar.activation(out=msq, in_=msq, func=AF.Sqrt,
                                         bias=eps)
                    nc.vector.reciprocal(out=msq, in_=msq)          # rstd
                    nc.vector.tensor_tensor(out=mu, in0=mu, in1=msq,
                                            op=ALU.mult)            # mu*rstd
                    for d in range(DT):
                        sc2 = tp.tile([P, 512], F32, tag="lnsc")
                        nc.vector.tensor_tensor(out=sc2, in0=xsq[:, d, 0, :],
                                                in1=msq, op=ALU.mult)
                        nc.vector.tensor_tensor(out=nout[:, d, :], in0=sc2,
                                                in1=mu, op=ALU.subtract)

                # ---------------- LN1 over full T (2 chunks, re & im) -----------
                nrf = ap_.tile([P, DT, T], BF16, tag="nbig1")
                nif = ap_.tile([P, DT, T], BF16, tag="nbig2")

                def ln1(comp, ch):
                    src_d = xTr_d if comp == 0 else xTi_d
                    dst = nrf if comp == 0 else nif

                    def fetch(d, src_d=src_d, ch=ch):
                        xch = tp.tile([P, 512], F32, tag="xch", bufs=3,
                                      name="xch")
                        xv = src_d.ap().rearrange("(o p) t -> p o t", p=P)
                        dma(xch, xv[:, d, ch * 512:(ch + 1) * 512])
                        return xch
                    ln_chunk(fetch, dst[:, :, ch * 512:(ch + 1) * 512],
                             f"1c{comp}{ch}")

                # ---------------- projection helper -----------------------------
                def cgroups(mA, mB, mC):
                    """complex matmul groups: re=(A,nr),(C,ni); im=(B,nr),(A,ni)"""
                    return (((mA, 0), (mC, 1)), ((mB, 0), (mA, 1)))

                def run_group(ps_, pairs, rhs_re, rhs_im, mt, ch):
                    n = len(pairs) * DT
                    i = 0
                    for m_, which in pairs:
                        r_ = rhs_re if which == 0 else rhs_im
                        for d in range(DT):
                            nc.tensor.matmul(
                                ps_, m_[:, d, mt * P:(mt + 1) * P],
                                r_[:, d, ch * 512:(ch + 1) * 512],
                                start=(i == 0), stop=(i == n - 1))
                            i += 1

                def run_pair(ps1, ps2, mA, mB, mC, rre, rim, mt, ch):
                    """re=A.nr+C.ni -> ps1 ; im=B.nr+A.ni -> ps2, sharing
                    the A stationary between adjacent matmuls."""
                    cs = slice(ch * 512, (ch + 1) * 512)
                    ms = slice(mt * P, (mt + 1) * P)
                    for d in range(DT):
                        nc.tensor.matmul(ps1, mA[:, d, ms], rre[:, d, cs],
                                         start=(d == 0), stop=False)
                        nc.tensor.matmul(ps2, mA[:, d, ms], rim[:, d, cs],
                                         start=(d == 0), stop=False)
                    for d in range(DT):
                        nc.tensor.matmul(ps1, mC[:, d, ms], rim[:, d, cs],
                                         start=False, stop=(d == DT - 1))
                    for d in range(DT):
                        nc.tensor.matmul(ps2, mB[:, d, ms], rre[:, d, cs],
                                         start=False, stop=(d == DT - 1))

                def msec(pk, i, cols=512):
                    return pk[:, i * DT * cols:(i + 1) * DT * cols].rearrange(
                        "p (k n) -> p k n", k=DT)

                # ---------------- LN1 ch0 -> Q -> LN1 ch1 -----------------------
                wq = loadpack(wq_d, "wq")
                ln1(0, 0)
                ln1(1, 0)

                # Q (own half = chunk 0) + stacks, per dtile
                qa, qb_, qc = msec(wq, 0), msec(wq, 1), msec(wq, 2)
                QS = []   # (QC_h, QD_h) per head
                gre, gim = cgroups(qa, qb_, qc)
                for d in range(DT):
                    qre_t = atp.tile([P, 512], BF16, tag="qp", bufs=4, name="qre")
                    qim_t = atp.tile([P, 512], BF16, tag="qp", bufs=4, name="qim")
                    nqim_t = atp.tile([P, 512], BF16, tag="qp", bufs=4, name="nqim")
                    ps1 = psp.tile([P, 512], F32, tag="mm", name="psq1")
                    run_group(ps1, gre, nrf, nif, d, 0)
                    nc.scalar.activation(out=qre_t, in_=ps1, func=AF.Identity,
                                         bias=b_qre[:, d:d + 1])
                    ps2 = psp.tile([P, 512], F32, tag="mm", name="psq2")
                    run_group(ps2, gim, nrf, nif, d, 0)
                    nc.scalar.activation(out=qim_t, in_=ps2, func=AF.Identity,
                                         bias=b_qim[:, d:d + 1])
                    nc.scalar.activation(out=nqim_t, in_=ps2, func=AF.Identity,
                                         scale=-1.0, bias=b_nqim[:, d:d + 1])
                    for hh in range(2):
                        h = 2 * d + hh
                        qc_h = atp.tile([P, 512], BF16, tag="qs", bufs=16,
                                        name=f"qc{h}")
                        qd_h = atp.tile([P, 512], BF16, tag="qs", bufs=16,
                                        name=f"qd{h}")
                        sl = slice(hh * DH, hh * DH + DH)
                        dma(qc_h[0:DH, :], qre_t[sl, :])
                        dma(qc_h[DH:P, :], nqim_t[sl, :])
                        dma(qd_h[0:DH, :], qim_t[sl, :])
                        dma(qd_h[DH:P, :], qre_t[sl, :])
                        QS.append((qc_h, qd_h))

                wka0 = loadpack(wka0_d, "wka0")
                wka1 = loadpack(wka1_d, "wka1")
                ln1(0, 1)
                ln1(1, 1)

                # ---------------- KA per head (full T) ---------------------------
                KAh = []
                for h in range(H):
                    pk = wka0 if h < 4 else wka1
                    hl = h % 4
                    kaA = pk[:, 0:2048].rearrange("p (k n) -> p k n", k=DT)
                    kaB = pk[:, 2048:4096].rearrange("p (k n) -> p k n", k=DT)
                    ka_h = atp.tile([P, T], BF16, tag="kah", bufs=8, name=f"ka{h}")
                    for ch in range(T // 512):
                        ps_ = psp.tile([P, 512], F32, tag="mm", name="psk")
                        i = 0
                        for m_, r_ in ((kaA, nrf), (kaB, nif)):
                            for d in range(DT):
                                nc.tensor.matmul(
                                    ps_, m_[:, d, hl * P:(hl + 1) * P],
                                    r_[:, d, ch * 512:(ch + 1) * 512],
                                    start=(i == 0), stop=(i == 7))
                                i += 1
                        nc.scalar.activation(
                            out=ka_h[:, ch * 512:(ch + 1) * 512], in_=ps_,
                            func=AF.Identity, bias=b_ka[:, h:h + 1])
                    KAh.append(ka_h)

                # ---------------- V token-major (stacked [vre|vim] per head) ----
                wvaR = loadpack(wvaR_d, "wvaR")
                wvaI = loadpack(wvaI_d, "wvaI", q=1)
                vR = wvaR[:, 0:4096].rearrange("p (k n) -> p k n", k=DT)
                vI = wvaI[:, 0:4096].rearrange("p (k n) -> p k n", k=DT)
                VA = ap_.tile([P, T // P, 2 * D], BF16, tag="VAx")
                VB = ap_.tile([P, T // P, 2 * D], BF16, tag="VBx")
                VAv = VA.rearrange("p k (h c) -> p k h c", c=P)
                VBv = VB.rearrange("p k (h c) -> p k h c", c=P)
                for tt in range(T // P):
                    tsl = slice(tt * P, (tt + 1) * P)
                    for hf in range(2):
                        csl = slice(hf * 512, (hf + 1) * 512)
                        ps_ = psp.tile([P, 512], F32, tag="mm", name="psv")
                        i = 0
                        for kt in range(DT):
                            for act, m_ in ((nrf, vR), (nif, vI)):
                                nc.tensor.matmul(
                                    ps_, act[:, kt, tsl], m_[:, kt, csl],
                                    start=(i == 0), stop=(i == 7))
                                i += 1
                        nc.scalar.activation(out=VA[:, tt, csl], in_=ps_,
                                             func=AF.Copy)
                    # VB = [-vim | vre] per head, strided from VA
                    nc.scalar.activation(out=VBv[:, tt, :, 0:DH],
                                         in_=VAv[:, tt, :, DH:P],
                                         func=AF.Copy, scale=-1.0)
                    nc.vector.tensor_copy(out=VBv[:, tt, :, DH:P],
                                          in_=VAv[:, tt, :, 0:DH])

                # ---------------- attention per head ----------------------------
                wo = loadpack(wo_d, "wo", q=1)
                or2_re = ap_.tile([P, DT, TQ], BF16, tag="bfa")
                or2_im = ap_.tile([P, DT, TQ], BF16, tag="bfb")
                KT = T // P
                for h in range(H):
                    qc_h, qd_h = QS[h]
                    ka_h = KAh[h]
                    ps_re = psa.tile([P, 512], F32, tag="av", name="psre")
                    ps_im = psa.tile([P, 512], F32, tag="av", name="psim")
                    zbp = pst.tile([P, 2, 512], F32,
                                   tag=("st" if h % 2 else "lnv"), name="zbp")
                    zbs = atp.tile([P, 2, 512], F32, tag="zbs", bufs=1,
                                   name="zbs")
                    pr1 = atp.tile([P, 512], BF16, tag="prl", bufs=4,
                                   name="pr1")
                    pr2 = atp.tile([P, 512], BF16, tag="prl", bufs=4,
                                   name="pr2")
                    t1 = tp.tile([P, 512], BF16, tag="cmb", name="t1")
                    t2 = tp.tile([P, 512], BF16, tag="cmb", name="t2")
                    for comp, (Qs_, ps_av, Vs) in enumerate(
                            ((qc_h, ps_re, VAv), (qd_h, ps_im, VBv))):
                        anT = atp.tile([P, KT, 512], BF16, tag="anT", bufs=2,
                                       name=f"anT{h}{comp}")
                        for kt in range(KT):
                            ps_ = psp.tile([P, 512], F32, tag="mm", name="pss")
                            nc.tensor.matmul(
                                ps_, ka_h[:, kt * P:(kt + 1) * P], Qs_,
                                start=True, stop=True)
                            nc.scalar.activation(
                                out=anT[:, kt, :], in_=ps_, func=AF.Exp,
                                bias=m10[:, 0:1])
                        for kt in range(KT):
                            nc.tensor.matmul(zbp[:, comp, :], onesw,
                                             anT[:, kt, :],
                                             start=(kt == 0), stop=(kt == KT - 1))
                        nc.vector.reciprocal(out=zbs[:, comp, :],
                                             in_=zbp[:, comp, :])
                        for kt in range(KT):
                            nc.tensor.matmul(
                                ps_av, Vs[:, kt, h, :], anT[:, kt, :],
                                start=(kt == 0),
                                stop=(kt == KT - 1))
                        pr = pr1 if comp == 0 else pr2
                        nc.scalar.activation(out=pr, in_=ps_av, func=AF.Copy)
                        nc.vector.tensor_tensor(
                            out=(t1 if comp == 0 else t2), in0=pr,
                            in1=zbs[:, comp, :], op=ALU.mult)
                    t3 = tp.tile([P, TQ], BF16, tag="cmb3", name="t3")
                    nc.vector.tensor_tensor(out=t3, in0=t1, in1=t2,
                                            op=ALU.add)
                    otmp = atp.tile([P, TQ], BF16, tag="otmp", name="otmp")
                    nc.scalar.activation(out=otmp, in_=t3, func=AF.Identity,
                                         bias=b_av[:, h:h + 1])
                    dsl = slice((h % 2) * DH, (h % 2) * DH + DH)
                    dma(or2_re[dsl, h // 2, :], otmp[0:DH, :])
                    dma(or2_im[dsl, h // 2, :], otmp[DH:P, :])

                # ---------------- o-proj (gated) + residual ---------------------
                oa, ob, oc = msec(wo, 0), msec(wo, 1), msec(wo, 2)
                x2r = ap_.tile([P, DT, TQ], F32, tag="VAx")
                x2i = ap_.tile([P, DT, TQ], F32, tag="VBx")
                og_re, og_im = cgroups(oa, ob, oc)
                n2r = ap_.tile([P, DT, TQ], BF16, tag="bfa")
                n2i = ap_.tile([P, DT, TQ], BF16, tag="bfb")

                def oproj(gi):
                    grp = og_re if gi == 0 else og_im
                    bias = b_ore if gi == 0 else b_oim
                    dst = x2r if gi == 0 else x2i
                    src_d = xTr_d if gi == 0 else xTi_d
                    xv = src_d.ap().rearrange("(o p) t -> p o t", p=P)
                    for mt in range(DT):
                        ps_ = psp.tile([P, 512], F32, tag="mm", name="pso")
                        run_group(ps_, grp, or2_re, or2_im, mt, 0)
                        og = tp.tile([P, TQ], F32, tag="og", name="og")
                        nc.scalar.activation(out=og, in_=ps_, func=AF.Identity,
                                             bias=bias[:, mt:mt + 1])
                        xres = tp.tile([P, TQ], F32, tag="xch", bufs=3,
                                       name="xres")
                        dma(xres, xv[:, mt, 0:TQ])
                        nc.vector.tensor_tensor(out=dst[:, mt, :], in0=og,
                                                in1=xres, op=ALU.add)

                # o-proj re -> LN2 re (DVE) overlaps o-proj im (tensor)
                oproj(0)
                ln_chunk(lambda d: x2r[:, d, :], n2r, "2r")
                oproj(1)
                ln_chunk(lambda d: x2i[:, d, :], n2i, "2i")

                g1r = ap_.tile([P, MLP // P, TQ], BF16, tag="nbig1")
                g1i = ap_.tile([P, MLP // P, TQ], BF16, tag="nbig2")
                for j in range(4):
                    pk = loadpack(wf1_d[j], f"wf1_{j}", q=j % 2)
                    f1a, f1b, f1c = msec(pk, 0), msec(pk, 1), msec(pk, 2)
                    fre, fim = cgroups(f1a, f1b, f1c)
                    if j == 0:
                        # phase-split: all (A, n2r) first so f1 starts
                        # before LN2-im finishes
                        pss = [psp.tile([P, 512], F32, tag="mm", name="psf1a"),
                               psp.tile([P, 512], F32, tag="mm", name="psf1b"),
                               psa.tile([P, 512], F32, tag="av", name="psf1c"),
                               psa.tile([P, 512], F32, tag="av", name="psf1d")]
                        for ml in range(4):
                            for d in range(DT):
                                nc.tensor.matmul(
                                    pss[ml], f1a[:, d, ml * P:(ml + 1) * P],
                                    n2r[:, d, :], start=(d == 0), stop=False)
                        for ml in range(4):
                            for d in range(DT):
                                nc.tensor.matmul(
                                    pss[ml], f1c[:, d, ml * P:(ml + 1) * P],
                                    n2i[:, d, :], start=False,
                                    stop=(d == DT - 1))
                            nc.scalar.activation(out=g1r[:, ml, :],
                                                 in_=pss[ml],
                                                 func=AF.Gelu_apprx_tanh,
                                                 bias=b_f1re[:, ml:ml + 1])
                        groups = ((1, fim),)
                    else:
                        groups = ((0, fre), (1, fim))
                    for gi, grp in groups:
                        bias = b_f1re if gi == 0 else b_f1im
                        dst = g1r if gi == 0 else g1i
                        for ml in range(4):
                            mt = j * 4 + ml
                            ps_ = psp.tile([P, 512], F32, tag="mm", name="psf1")
                            run_group(ps_, grp, n2r, n2i, ml, 0)
                            nc.scalar.activation(out=dst[:, mt, :], in_=ps_,
                                                 func=AF.Gelu_apprx_tanh,
                                                 bias=bias[:, mt:mt + 1])

                # f2: single pack sweep, all 8 psum banks held
                # (LN's st/lnv psum tags are free by now)
                ps_st = pst.tile([P, 2, 512], F32, tag="st", name="pf2st")
                ps_lnv = pst.tile([P, 2, 512], F32, tag="lnv", name="pf2lnv")
                psums = [psp.tile([P, 512], F32, tag="mm", name="pf2a"),
                         psp.tile([P, 512], F32, tag="mm", name="pf2b"),
                         psa.tile([P, 512], F32, tag="av", name="pf2c"),
                         psa.tile([P, 512], F32, tag="av", name="pf2d"),
                         ps_st[:, 0, :], ps_st[:, 1, :],
                         ps_lnv[:, 0, :], ps_lnv[:, 1, :]]
                for j in range(4):
                    pk = loadpack(wf2_d[j], f"wf2p_{j}", q=j % 2)
                    f2a, f2b, f2c = msec(pk, 0), msec(pk, 1), msec(pk, 2)
                    for gi in range(2):
                        pairs = ((f2a, 0), (f2c, 1)) if gi == 0 else \
                                ((f2b, 0), (f2a, 1))
                        for mt in range(DT):
                            i = 0
                            for m_, which in pairs:
                                r_ = g1r if which == 0 else g1i
                                for kl in range(4):
                                    nc.tensor.matmul(
                                        psums[gi * 4 + mt],
                                        m_[:, kl, mt * P:(mt + 1) * P],
                                        r_[:, j * 4 + kl, :],
                                        start=(j == 0 and i == 0),
                                        stop=(j == 3 and i == 7))
                                    i += 1
                ov = out_d.ap().rearrange("c (o p) t -> c p o t", p=P)
                for gi in range(2):
                    bias = b_f2re if gi == 0 else b_f2im
                    x2s = x2r if gi == 0 else x2i
                    for mt in range(DT):
                        fg = tp.tile([P, TQ], F32, tag="og", name="fg")
                        nc.scalar.activation(out=fg, in_=psums[gi * 4 + mt],
                                             func=AF.Identity,
                                             bias=bias[:, mt:mt + 1])
                        oc_ = tp.tile([P, TQ], F32, tag="outc", bufs=2, name="oc")
                        nc.vector.tensor_tensor(out=oc_, in0=fg,
                                                in1=x2s[:, mt, :], op=ALU.add)
                        dma(ov[gi, :, mt, :], oc_)


            for _rep in range(reps):
                emit()

    _split_dma_waits(nc)
    return nc


def _split_dma_waits(nc):
    """Walrus's DIRECT2D DMA encoding takes one sync wait; move extra
    waits onto a preceding sequencer EventSemaphore on the same engine."""
    f = nc.m.functions[0]
    for blk in f.blocks:
        out = []
        for ins in blk.instructions:
            si = getattr(ins, 'sync_info', None)
            tn = type(ins).__name__
            lim = 1
            if si is not None and si.on_wait and len(si.on_wait) > lim:
                waits = list(si.on_wait)
                extra = waits[:-lim]
                si.on_wait = waits[-lim:]
                k = 0
                while extra:
                    ev = mybir.InstEventSemaphore(
                        name=f"{ins.name}_wsplit{k}",
                        engine=ins.engine,
                        ins=[], outs=[],
                        sync_info=mybir.SyncInfo(on_wait=extra[:2],
                                                 on_update=[]),
                    )
                    out.append(ev)
                    extra = extra[2:]
                    k += 1
            out.append(ins)
        blk.instructions = out


_NC_CACHE = None


def _get_nc():
    global _NC_CACHE
    if _NC_CACHE is None:
        _NC_CACHE = build_nc()
    return _NC_CACHE


TRACE = False
LAST_RESULT = None


def kernel(**inputs):
    global LAST_RESULT
    nc = _get_nc()
    in_maps = []
    for c in range(NCORES):
        in_maps.append(_prep_core(inputs, c // 2, c % 2))
    res = run_bass_kernel_spmd(nc, in_maps, list(range(NCORES)),
                               trace=TRACE)
    LAST_RESULT = res
    out = np.empty((2, B, T, D), np.float32)
    for c in range(NCORES):
        b, half = c // 2, c % 2
        y = res.results[c]["outT"]          # [2, D, TQ]
        out[:, b, half * TQ:(half + 1) * TQ, :] = y.transpose(0, 2, 1)
    return out


# revision 39
# speedup vs baseline: 1.0730x; 1.0730x over previous
"""CDiT block kernel for 8 TRN2 NeuronCores.

Sharding: core c handles batch b=c//2, token half h=c%2 (512 of 1024 tokens).
Each core computes the full output for its (b, token-half) slice; K/V are
computed over the full T of the batch (duplicated within the pair), so no
cross-core collectives are needed.

Host folds adaLN modulation (scale/shift) and gates into the projection
weights/biases (they are per-batch constants), pre-transposes weights to
[din, dout] bf16, and pre-transposes x to feature-major [D, T] with the
token axis rolled so each core's own tokens are [0:512].

Device layout: activations feature-major [128, d_tiles, T]; matmuls use
weights as stationary lhsT and activations as moving rhs (feature-major out).
V is produced token-major directly (activation tiles stationary, weight
columns moving) in the stacked per-head [vre|vim] layout, so AV needs no
DMA transposes. Scores are computed transposed (S^T, k on partitions) with
stacked-contraction K/Q ([Kr;Ki] etc.), so exp(S^T) feeds AV directly as
the moving operand; softmax denominators come from ones-matmuls, the 1/z
normalization is broadcast across partitions via a ones-column matmul, and
the V bias is folded in with a rank-1 (b x z) matmul before normalizing.
Softmax has no max subtraction (scores are O(1); a fixed -10 bias guards
overflow).
"""

import os
import sys
import numpy as np

for _p in ("/opt/trn_rl_repo",):
    if _p not in sys.path:
        sys.path.insert(0, _p)

import ml_dtypes

import concourse.bass as bass
import concourse.mybir as mybir
import concourse.tile as tile
from concourse.bass_utils import run_bass_kernel_spmd

B, T, D, H = 4, 1024, 512, 8
DH = D // H
MLP = 4 * D
EPS = 1e-6
P = 128
DT = D // P          # 4 feature tiles
TQ = T // 2          # own tokens per core
NCORES = 8

F32 = mybir.dt.float32
BF16 = mybir.dt.bfloat16
FP8 = mybir.dt.float8e4
AF = mybir.ActivationFunctionType
ALU = mybir.AluOpType
BF = ml_dtypes.bfloat16


# ----------------------------------------------------------------------------
# Host-side prep
# ----------------------------------------------------------------------------

def _feat_major(w_t):
    """[din, dout] -> [128, din//128 * dout] with din = kt*128 + p."""
    din, dout = w_t.shape
    return np.ascontiguousarray(
        w_t.reshape(din // P, P, dout).transpose(1, 0, 2).reshape(P, -1)
    )


def _col(v):
    """[dout] -> [128, dout//128] per-partition bias layout (d = o*128+p)."""
    return np.ascontiguousarray(v.reshape(-1, P).T)


def _silu(x):
    return x / (1.0 + np.exp(-x))


def _prep_core(inputs, b, half):
    """Build the per-core input map (numpy arrays, host precomputation)."""
    f = np.float32
    g = lambda n: np.asarray(inputs[n], dtype=f)

    # adaLN on host (tiny): complex silu -> complex linear -> 6 chunks
    sr, si = _silu(g('c_re')[b]), _silu(g('c_im')[b])
    aWr, aWi = g('ada_Wr'), g('ada_Wi')
    m_re = aWr @ sr - aWi @ si + (g('ada_br') - g('ada_bi'))
    m_im = aWr @ si + aWi @ sr + (g('ada_br') + g('ada_bi'))
    sh_ar, sc_ar, g_ar, sh_mr, sc_mr, g_mr = np.split(m_re, 6)
    sh_ai, sc_ai, g_ai, sh_mi, sc_mi, g_mi = np.split(m_im, 6)

    def fold_mod(Wr, Wi, br, bi, a, bb, shr, shi):
        """Fold complex modulate diag(a+ib)+shift into complex linear."""
        Mr = Wr * a[None, :] - Wi * bb[None, :]
        Mi = Wi * a[None, :] + Wr * bb[None, :]
        bre = (br - bi) + Wr @ shr - Wi @ shi
        bim = (br + bi) + Wi @ shr + Wr @ shi
        return Mr, Mi, bre, bim

    a1, b1 = 1.0 + sc_ar, sc_ai
    a2, b2 = 1.0 + sc_mr, sc_mi

    qMr, qMi, qbre, qbim = fold_mod(g('q_Wr'), g('q_Wi'), g('q_br'), g('q_bi'),
                                    a1, b1, sh_ar, sh_ai)
    kMr, kMi, kbre, kbim = fold_mod(g('k_Wr'), g('k_Wi'), g('k_br'), g('k_bi'),
                                    a1, b1, sh_ar, sh_ai)
    vMr, vMi, vbre, vbim = fold_mod(g('v_Wr'), g('v_Wi'), g('v_br'), g('v_bi'),
                                    a1, b1, sh_ar, sh_ai)
    scale = 1.0 / np.sqrt(np.float32(DH))
    qMr, qMi, qbre, qbim = qMr * scale, qMi * scale, qbre * scale, qbim * scale

    f1Mr, f1Mi, f1bre, f1bim = fold_mod(g('f1_Wr'), g('f1_Wi'),
                                        g('f1_br'), g('f1_bi'),
                                        a2, b2, sh_mr, sh_mi)

    # o-proj with attention gate folded (row scaling by complex g_a)
    oWr, oWi = g('o_Wr'), g('o_Wi')
    oGr = g_ar[:, None] * oWr - g_ai[:, None] * oWi
    oGi = g_ai[:, None] * oWr + g_ar[:, None] * oWi
    obre, obim = g('o_br') - g('o_bi'), g('o_br') + g('o_bi')
    ogbre = g_ar * obre - g_ai * obim
    ogbim = g_ai * obre + g_ar * obim

    # f2 with MLP gate folded
    fWr, fWi = g('f2_Wr'), g('f2_Wi')
    fGr = g_mr[:, None] * fWr - g_mi[:, None] * fWi
    fGi = g_mi[:, None] * fWr + g_mr[:, None] * fWi
    fbre, fbim = g('f2_br') - g('f2_bi'), g('f2_br') + g('f2_bi')
    fgbre = g_mr * fbre - g_mi * fbim
    fgbim = g_mi * fbre + g_mr * fbim

    # KA stacked weights: out rows = per head [Kr_h(64); Ki_h(64)]
    kA = np.empty((D * 2, D), f)   # rows for nr
    kB = np.empty((D * 2, D), f)   # rows for ni
    ka_b = np.empty(D * 2, f)
    for h in range(H):
        r = slice(h * DH, (h + 1) * DH)
        blk = slice(h * P, h * P + DH)
        blk2 = slice(h * P + DH, (h + 1) * P)
        kA[blk], kA[blk2] = kMr[r], kMi[r]
        kB[blk], kB[blk2] = -kMi[r], kMr[r]
        ka_b[blk], ka_b[blk2] = kbre[r], kbim[r]

    # V moving packs for token-major V^T: cols c = h*128 + (re 64 | im 64)
    WVAr = np.empty((D, 2 * D), f)   # moving for nr
    WVAi = np.empty((D, 2 * D), f)   # moving for ni
    for h in range(H):
        r = slice(h * DH, (h + 1) * DH)
        WVAr[:, h * P: h * P + DH] = vMr[r].T
        WVAr[:, h * P + DH: (h + 1) * P] = vMi[r].T
        WVAi[:, h * P: h * P + DH] = -vMi[r].T
        WVAi[:, h * P + DH: (h + 1) * P] = vMr[r].T

    # AV bias row (folded post-normalization via b*z rank-1 matmul):
    # [bvre-bvim; bvre+bvim] per head
    av_b = np.empty(D * 2, f)
    for h in range(H):
        r = slice(h * DH, (h + 1) * DH)
        av_b[h * P: h * P + DH] = vbre[r] - vbim[r]
        av_b[h * P + DH: (h + 1) * P] = vbre[r] + vbim[r]
    av_row = np.ascontiguousarray(av_b[None, :].astype(BF))   # [1, 1024]

    bf = lambda w: _feat_major(w).astype(BF)

    wq = np.concatenate([bf(qMr.T), bf(qMi.T), bf(-qMi.T)], axis=1)
    wka0 = np.concatenate([bf(kA.T[:, 0:512]), bf(kB.T[:, 0:512])], axis=1)
    wka1 = np.concatenate([bf(kA.T[:, 512:1024]), bf(kB.T[:, 512:1024])],
                          axis=1)
    wvaR = bf(WVAr)
    wvaI = bf(WVAi)
    wo = np.concatenate([bf(oGr.T), bf(oGi.T), bf(-oGi.T)], axis=1)
    wf1 = [np.concatenate([bf(f1Mr.T[:, j * 512:(j + 1) * 512]),
                           bf(f1Mi.T[:, j * 512:(j + 1) * 512]),
                           bf(-f1Mi.T[:, j * 512:(j + 1) * 512])], axis=1)
           for j in range(4)]
    wf2 = [np.concatenate([bf(fGr.T[j * 512:(j + 1) * 512]),
                           bf(fGi.T[j * 512:(j + 1) * 512]),
                           bf(-fGi.T[j * 512:(j + 1) * 512])], axis=1)
           for j in range(4)]

    smalls = np.concatenate([
        _col(qbre), _col(qbim), _col(-qbim),           # 0:4, 4:8, 8:12
        _col(ka_b),                                    # 12:20
        _col(ogbre), _col(ogbim),                      # 20:24, 24:28
        _col(f1bre), _col(f1bim),                      # 28:44, 44:60
        _col(fgbre), _col(fgbim),                      # 60:64, 64:68
        np.full((P, 1), EPS, f),                       # 68
    ], axis=1)

    roll = lambda a: np.roll(a, -half * TQ, axis=0)
    xTr = np.ascontiguousarray(roll(g('x_re')[b]).T)
    xTi = np.ascontiguousarray(roll(g('x_im')[b]).T)

    im = {'xTr': xTr, 'xTi': xTi, 'wq': wq, 'wka0': wka0, 'wka1': wka1,
          'wvaR': wvaR, 'wvaI': wvaI, 'wo': wo, 'smalls': smalls,
          'avrow': av_row}
    for j in range(4):
        im[f'wf1_{j}'] = wf1[j]
        im[f'wf2_{j}'] = wf2[j]
    return im


# ----------------------------------------------------------------------------
# Device program
# ----------------------------------------------------------------------------

def build_nc(reps=1):
    nc = bass.Bass()

    xTr_d = nc.declare_dram_parameter("xTr", [D, T], F32, isOutput=False)
    xTi_d = nc.declare_dram_parameter("xTi", [D, T], F32, isOutput=False)
    wq_d = nc.declare_dram_parameter("wq", [P, 6144], BF16, isOutput=False)
    wka0_d = nc.declare_dram_parameter("wka0", [P, 4096], BF16, isOutput=False)
    wka1_d = nc.declare_dram_parameter("wka1", [P, 4096], BF16, isOutput=False)
    wvaR_d = nc.declare_dram_parameter("wvaR", [P, 4096], BF16, isOutput=False)
    wvaI_d = nc.declare_dram_parameter("wvaI", [P, 4096], BF16, isOutput=False)
    wo_d = nc.declare_dram_parameter("wo", [P, 6144], BF16, isOutput=False)
    wf1_d = [nc.declare_dram_parameter(f"wf1_{j}", [P, 6144], BF16,
                                       isOutput=False) for j in range(4)]
    wf2_d = [nc.declare_dram_parameter(f"wf2_{j}", [P, 6144], BF16,
                                       isOutput=False) for j in range(4)]
    smalls_d = nc.declare_dram_parameter("smalls", [P, 69], F32, isOutput=False)
    avrow_d = nc.declare_dram_parameter("avrow", [1, 2 * D], BF16,
                                        isOutput=False)
    out_d = nc.declare_dram_parameter("outT", [2, D, TQ], F32, isOutput=True)

    with tile.TileContext(nc) as tc:
        with (
            tc.tile_pool(name="persist", bufs=1) as pp,
            tc.tile_pool(name="acts", bufs=1) as ap_,
            tc.tile_pool(name="tmp", bufs=2) as tp,
            tc.tile_pool(name="attn", bufs=2) as atp,
            tc.tile_pool(name="psum", bufs=2, space="PSUM") as psp,
            tc.tile_pool(name="psum_av", bufs=2, space="PSUM") as psa,
            tc.tile_pool(name="psum_st", bufs=1, space="PSUM") as pst,
        ):
            def emit():
                dma = nc.sync.dma_start

                smalls = pp.tile([P, 69], F32)
                dma(smalls, smalls_d.ap())
                b_qre, b_qim, b_nqim = smalls[:, 0:4], smalls[:, 4:8], smalls[:, 8:12]
                b_ka = smalls[:, 12:20]
                b_ore, b_oim = smalls[:, 20:24], smalls[:, 24:28]
                b_f1re, b_f1im = smalls[:, 28:44], smalls[:, 44:60]
                b_f2re, b_f2im = smalls[:, 60:64], smalls[:, 64:68]
                eps = smalls[:, 68:69]

                avrow = pp.tile([1, 2 * D], BF16)
                dma(avrow, avrow_d.ap())

                ones = pp.tile([P, 1], BF16)
                nc.vector.memset(ones, 1.0)
                onesw = pp.tile([P, P], BF16)
                nc.vector.memset(onesw, 1.0)
                oneD = pp.tile([P, 1], BF16)
                nc.vector.memset(oneD, 1.0 / D)
                onesb = pp.tile([1, P], BF16)
                nc.vector.memset(onesb, 1.0)
                m10 = pp.tile([P, 1], F32)
                nc.vector.memset(m10, -10.0)

                def loadpack(src, n, q=0):
                    wpk = pp.tile([P, 6144], BF16, tag="wpk", bufs=2, name=n)
                    eng = nc.sync if q == 0 else nc.gpsimd
                    eng.dma_start(wpk[:, 0:src.shape[1]], src.ap())
                    return wpk

                # ---------------- LayerNorm helper (per 512-token chunk) --------
                def ln_chunk(fetch, nout, tag):
                    """fetch(d) -> [P, 512] f32 AP; nout [P, DT, 512] bf16."""
                    xsq = tp.tile([P, DT, 2, 512], BF16, tag="xsq", bufs=2)
                    for d in range(DT):
                        xf = fetch(d)
                        nc.scalar.activation(out=xsq[:, d, 0, :], in_=xf,
                                             func=AF.Copy)
                        nc.scalar.activation(out=xsq[:, d, 1, :], in_=xf,
                                             func=AF.Square)
                    ps = pst.tile([1, 2, 512], F32, tag="st")
                    for s in range(2):
                        for d in range(DT):
                            nc.tensor.matmul(ps[:, s, :], oneD[:, 0:1],
                                             xsq[:, d, s, :],
                                             start=(d == 0), stop=(d == DT - 1))
                    strow = tp.tile([1, 2, 512], BF16, tag="strow", bufs=1)
                    nc.scalar.activation(out=strow, in_=ps, func=AF.Copy)
                    lnvp = pst.tile([P, 2, 512], F32, tag="lnv")
                    for s in range(2):
                        nc.tensor.matmul(lnvp[:, s, :], onesb, strow[0:1, s, :],
                                         start=True, stop=True)
                    lnv = tp.tile([P, 2, 512], F32, tag="lnv", bufs=1)
                    nc.scalar.activation(out=lnv, in_=lnvp, func=AF.Copy)
                    mu, msq = lnv[:, 0, :], lnv[:, 1, :]
                    sc = tp.tile([P, 512], F32, tag="lnsc")
                    nc.vector.tensor_tensor(out=sc, in0=mu, in1=mu, op=ALU.mult)
                    nc.vector.tensor_tensor(out=msq, in0=msq, in1=sc,
                                            op=ALU.subtract)
                    nc.scalar.activation(out=msq, in_=msq, func=AF.Sqrt,
                                         bias=eps)
                    nc.vector.reciprocal(out=msq, in_=msq)          # rstd
                    nc.vector.tensor_tensor(out=mu, in0=mu, in1=msq,
                                            op=ALU.mult)            # mu*rstd
                    for d in range(DT):
                        sc2 = tp.tile([P, 512], F32, tag="lnsc")
                        nc.vector.tensor_tensor(out=sc2, in0=xsq[:, d, 0, :],
                                                in1=msq, op=ALU.mult)
                        nc.vector.tensor_tensor(out=nout[:, d, :], in0=sc2,
                                                in1=mu, op=ALU.subtract)

                # ---------------- LN1 over full T (2 chunks, re & im) -----------
                nrf = ap_.tile([P, DT, T], BF16, tag="nbig1")
                nif = ap_.tile([P, DT, T], BF16, tag="nbig2")

                def ln1(comp, ch):
                    src_d = xTr_d if comp == 0 else xTi_d
                    dst = nrf if comp == 0 else nif

                    def fetch(d, src_d=src_d, ch=ch):
                        xch = tp.tile([P, 512], F32, tag="xch", bufs=3,
                                      name="xch")
                        xv = src_d.ap().rearrange("(o p) t -> p o t", p=P)
                        dma(xch, xv[:, d, ch * 512:(ch + 1) * 512])
                        return xch
                    ln_chunk(fetch, dst[:, :, ch * 512:(ch + 1) * 512],
                             f"1c{comp}{ch}")

                # ---------------- projection helper -----------------------------
                def cgroups(mA, mB, mC):
                    """complex matmul groups: re=(A,nr),(C,ni); im=(B,nr),(A,ni)"""
                    return (((mA, 0), (mC, 1)), ((mB, 0), (mA, 1)))

                def run_group(ps_, pairs, rhs_re, rhs_im, mt, ch):
                    n = len(pairs) * DT
                    i = 0
                    for m_, which in pairs:
                        r_ = rhs_re if which == 0 else rhs_im
                        for d in range(DT):
                            nc.tensor.matmul(
                                ps_, m_[:, d, mt * P:(mt + 1) * P],
                                r_[:, d, ch * 512:(ch + 1) * 512],
                                start=(i == 0), stop=(i == n - 1))
                            i += 1

                def run_pair(ps1, ps2, mA, mB, mC, rre, rim, mt, ch):
                    """re=A.nr+C.ni -> ps1 ; im=B.nr+A.ni -> ps2, sharing
                    the A stationary between adjacent matmuls."""
                    cs = slice(ch * 512, (ch + 1) * 512)
                    ms = slice(mt * P, (mt + 1) * P)
                    for d in range(DT):
                        nc.tensor.matmul(ps1, mA[:, d, ms], rre[:, d, cs],
                                         start=(d == 0), stop=False)
                        nc.tensor.matmul(ps2, mA[:, d, ms], rim[:, d, cs],
                                         start=(d == 0), stop=False)
                    for d in range(DT):
                        nc.tensor.matmul(ps1, mC[:, d, ms], rim[:, d, cs],
                                         start=False, stop=(d == DT - 1))
                    for d in range(DT):
                        nc.tensor.matmul(ps2, mB[:, d, ms], rre[:, d, cs],
                                         start=False, stop=(d == DT - 1))

                def msec(pk, i, cols=512):
                    return pk[:, i * DT * cols:(i + 1) * DT * cols].rearrange(
                        "p (k n) -> p k n", k=DT)

                # ---------------- LN1 ch0 -> Q -> LN1 ch1 -----------------------
                wq = loadpack(wq_d, "wq")
                ln1(0, 0)
                ln1(1, 0)

                # Q (own half = chunk 0) + stacks, per dtile
                qa, qb_, qc = msec(wq, 0), msec(wq, 1), msec(wq, 2)
                QS = []   # (QC_h, QD_h) per head
                gre, gim = cgroups(qa, qb_, qc)
                for d in range(DT):
                    qre_t = atp.tile([P, 512], BF16, tag="qp", bufs=4, name="qre")
                    qim_t = atp.tile([P, 512], BF16, tag="qp", bufs=4, name="qim")
                    nqim_t = atp.tile([P, 512], BF16, tag="qp", bufs=4, name="nqim")
                    ps1 = psp.tile([P, 512], F32, tag="mm", name="psq1")
                    run_group(ps1, gre, nrf, nif, d, 0)
                    nc.scalar.activation(out=qre_t, in_=ps1, func=AF.Identity,
                                         bias=b_qre[:, d:d + 1])
                    ps2 = psp.tile([P, 512], F32, tag="mm", name="psq2")
                    run_group(ps2, gim, nrf, nif, d, 0)
                    nc.scalar.activation(out=qim_t, in_=ps2, func=AF.Identity,
                                         bias=b_qim[:, d:d + 1])
                    nc.scalar.activation(out=nqim_t, in_=ps2, func=AF.Identity,
                                         scale=-1.0, bias=b_nqim[:, d:d + 1])
                    for hh in range(2):
                        h = 2 * d + hh
                        qc_h = atp.tile([P, 512], BF16, tag="qs", bufs=16,
                                        name=f"qc{h}")
                        qd_h = atp.tile([P, 512], BF16, tag="qs", bufs=16,
                                        name=f"qd{h}")
                        sl = slice(hh * DH, hh * DH + DH)
                        dma(qc_h[0:DH, :], qre_t[sl, :])
                        dma(qc_h[DH:P, :], nqim_t[sl, :])
                        dma(qd_h[0:DH, :], qim_t[sl, :])
                        dma(qd_h[DH:P, :], qre_t[sl, :])
                        QS.append((qc_h, qd_h))

                wka0 = loadpack(wka0_d, "wka0")
                wka1 = loadpack(wka1_d, "wka1")
                ln1(0, 1)
                ln1(1, 1)

                # ---------------- KA per head (full T) ---------------------------
                KAh = []
                for h in range(H):
                    pk = wka0 if h < 4 else wka1
                    hl = h % 4
                    kaA = pk[:, 0:2048].rearrange("p (k n) -> p k n", k=DT)
                    kaB = pk[:, 2048:4096].rearrange("p (k n) -> p k n", k=DT)
                    ka_h = atp.tile([P, T], BF16, tag="kah", bufs=8, name=f"ka{h}")
                    for ch in range(T // 512):
                        ps_ = psp.tile([P, 512], F32, tag="mm", name="psk")
                        i = 0
                        for m_, r_ in ((kaA, nrf), (kaB, nif)):
                            for d in range(DT):
                                nc.tensor.matmul(
                                    ps_, m_[:, d, hl * P:(hl + 1) * P],
                                    r_[:, d, ch * 512:(ch + 1) * 512],
                                    start=(i == 0), stop=(i == 7))
                                i += 1
                        nc.scalar.activation(
                            out=ka_h[:, ch * 512:(ch + 1) * 512], in_=ps_,
                            func=AF.Identity, bias=b_ka[:, h:h + 1])
                    KAh.append(ka_h)

                # ---------------- V token-major (stacked [vre|vim] per head) ----
                wvaR = loadpack(wvaR_d, "wvaR")
                wvaI = loadpack(wvaI_d, "wvaI", q=1)
                vR = wvaR[:, 0:4096].rearrange("p (k n) -> p k n", k=DT)
                vI = wvaI[:, 0:4096].rearrange("p (k n) -> p k n", k=DT)
                VA = ap_.tile([P, T // P, 2 * D], BF16, tag="VAx")
                VB = ap_.tile([P, T // P, 2 * D], BF16, tag="VBx")
                VAv = VA.rearrange("p k (h c) -> p k h c", c=P)
                VBv = VB.rearrange("p k (h c) -> p k h c", c=P)
                for tt in range(T // P):
                    tsl = slice(tt * P, (tt + 1) * P)
                    for hf in range(2):
                        csl = slice(hf * 512, (hf + 1) * 512)
                        ps_ = psp.tile([P, 512], F32, tag="mm", name="psv")
                        i = 0
                        for kt in range(DT):
                            for act, m_ in ((nrf, vR), (nif, vI)):
                                nc.tensor.matmul(
                                    ps_, act[:, kt, tsl], m_[:, kt, csl],
                                    start=(i == 0), stop=(i == 7))
                                i += 1
                        nc.scalar.activation(out=VA[:, tt, csl], in_=ps_,
                                             func=AF.Copy)
                    # VB = [-vim | vre] per head, strided from VA
                    nc.scalar.activation(out=VBv[:, tt, :, 0:DH],
                                         in_=VAv[:, tt, :, DH:P],
                                         func=AF.Copy, scale=-1.0)
                    nc.vector.tensor_copy(out=VBv[:, tt, :, DH:P],
                                          in_=VAv[:, tt, :, 0:DH])

                # ---------------- attention per head ----------------------------
                wo = loadpack(wo_d, "wo", q=1)
                or2_re = ap_.tile([P, DT, TQ], BF16, tag="bfa")
                or2_im = ap_.tile([P, DT, TQ], BF16, tag="bfb")
                KT = T // P
                for h in range(H):
                    qc_h, qd_h = QS[h]
                    ka_h = KAh[h]
                    ps_re = psa.tile([P, 512], F32, tag="av", name="psre")
                    ps_im = psa.tile([P, 512], F32, tag="av", name="psim")
                    zsb = atp.tile([1, 2, 512], BF16, tag="zsb", bufs=2,
                                   name="zsb")
                    zbp = pst.tile([P, 2, 512], F32,
                                   tag=("st" if h % 2 else "lnv"), name="zbp")
                    zbs = atp.tile([P, 2, 512], F32, tag="zbs", bufs=1,
                                   name="zbs")
                    pr1 = atp.tile([P, 512], BF16, tag="prl", bufs=4,
                                   name="pr1")
                    pr2 = atp.tile([P, 512], BF16, tag="prl", bufs=4,
                                   name="pr2")
                    t1 = tp.tile([P, 512], BF16, tag="cmb", name="t1")
                    t2 = tp.tile([P, 512], BF16, tag="cmb", name="t2")
                    for comp, (Qs_, ps_av, Vs) in enumerate(
                            ((qc_h, ps_re, VAv), (qd_h, ps_im, VBv))):
                        anT = atp.tile([P, KT, 512], BF16, tag="anT", bufs=2,
                                       name=f"anT{h}{comp}")
                        for kt in range(KT):
                            ps_ = psp.tile([P, 512], F32, tag="mm", name="pss")
                            nc.tensor.matmul(
                                ps_, ka_h[:, kt * P:(kt + 1) * P], Qs_,
                                start=True, stop=True)
                            nc.scalar.activation(
                                out=anT[:, kt, :], in_=ps_, func=AF.Exp,
                                bias=m10[:, 0:1])
                        for kt in range(KT):
                            nc.tensor.matmul(zbp[:, comp, :], onesw,
                                             anT[:, kt, :],
                                             start=(kt == 0), stop=(kt == KT - 1))
                        # z row -> sbuf for the bias rank-1; 1/z on DVE
                        nc.vector.tensor_copy(out=zsb[:, comp, :],
                                              in_=zbp[0:1, comp, :])
                        if comp == 0:
                            # fold V bias into ps_re as b_av[c]*z_re[q]
                            nc.tensor.matmul(ps_re,
                                             avrow[:, h * P:(h + 1) * P],
                                             zsb[0:1, 0, :],
                                             start=True, stop=False)
                        nc.vector.reciprocal(out=zbs[:, comp, :],
                                             in_=zbp[:, comp, :])
                        for kt in range(KT):
                            nc.tensor.matmul(
                                ps_av, Vs[:, kt, h, :], anT[:, kt, :],
                                start=(comp == 1 and kt == 0),
                                stop=(kt == KT - 1))
                        # release the AV psum via DVE (ACT is the
                        # attention pacer); fold the 1/z multiply in directly
                        nc.vector.tensor_tensor(
                            out=(t1 if comp == 0 else t2), in0=ps_av,
                            in1=zbs[:, comp, :], op=ALU.mult)
                    otmp = atp.tile([P, TQ], BF16, tag="otmp", name="otmp")
                    nc.vector.tensor_tensor(out=otmp, in0=t1, in1=t2,
                                            op=ALU.add)
                    dsl = slice((h % 2) * DH, (h % 2) * DH + DH)
                    dma(or2_re[dsl, h // 2, :], otmp[0:DH, :])
                    dma(or2_im[dsl, h // 2, :], otmp[DH:P, :])

                # ---------------- o-proj (gated) + residual ---------------------
                oa, ob, oc = msec(wo, 0), msec(wo, 1), msec(wo, 2)
                x2r = ap_.tile([P, DT, TQ], F32, tag="VAx")
                x2i = ap_.tile([P, DT, TQ], F32, tag="VBx")
                og_re, og_im = cgroups(oa, ob, oc)
                n2r = ap_.tile([P, DT, TQ], BF16, tag="bfa")
                n2i = ap_.tile([P, DT, TQ], BF16, tag="bfb")

                def oproj(gi):
                    grp = og_re if gi == 0 else og_im
                    bias = b_ore if gi == 0 else b_oim
                    dst = x2r if gi == 0 else x2i
                    src_d = xTr_d if gi == 0 else xTi_d
                    xv = src_d.ap().rearrange("(o p) t -> p o t", p=P)
                    for mt in range(DT):
                        ps_ = psp.tile([P, 512], F32, tag="mm", name="pso")
                        run_group(ps_, grp, or2_re, or2_im, mt, 0)
                        og = tp.tile([P, TQ], F32, tag="og", name="og")
                        nc.scalar.activation(out=og, in_=ps_, func=AF.Identity,
                                             bias=bias[:, mt:mt + 1])
                        xres = tp.tile([P, TQ], F32, tag="xch", bufs=3,
                                       name="xres")
                        dma(xres, xv[:, mt, 0:TQ])
                        nc.vector.tensor_tensor(out=dst[:, mt, :], in0=og,
                                                in1=xres, op=ALU.add)

                # o-proj re -> LN2 re (DVE) overlaps o-proj im (tensor)
                oproj(0)
                ln_chunk(lambda d: x2r[:, d, :], n2r, "2r")
                oproj(1)
                ln_chunk(lambda d: x2i[:, d, :], n2i, "2i")

                g1r = ap_.tile([P, MLP // P, TQ], BF16, tag="nbig1")
                g1i = ap_.tile([P, MLP // P, TQ], BF16, tag="nbig2")
                for j in range(4):
                    pk = loadpack(wf1_d[j], f"wf1_{j}", q=j % 2)
                    f1a, f1b, f1c = msec(pk, 0), msec(pk, 1), msec(pk, 2)
                    fre, fim = cgroups(f1a, f1b, f1c)
                    if j == 0:
                        # phase-split: all (A, n2r) first so f1 starts
                        # before LN2-im finishes
                        pss = [psp.tile([P, 512], F32, tag="mm", name="psf1a"),
                               psp.tile([P, 512], F32, tag="mm", name="psf1b"),
                               psa.tile([P, 512], F32, tag="av", name="psf1c"),
                               psa.tile([P, 512], F32, tag="av", name="psf1d")]
                        for ml in range(4):
                            for d in range(DT):
                                nc.tensor.matmul(
                                    pss[ml], f1a[:, d, ml * P:(ml + 1) * P],
                                    n2r[:, d, :], start=(d == 0), stop=False)
                        for ml in range(4):
                            for d in range(DT):
                                nc.tensor.matmul(
                                    pss[ml], f1c[:, d, ml * P:(ml + 1) * P],
                                    n2i[:, d, :], start=False,
                                    stop=(d == DT - 1))
                            nc.scalar.activation(out=g1r[:, ml, :],
                                                 in_=pss[ml],
                                                 func=AF.Gelu_apprx_tanh,
                                                 bias=b_f1re[:, ml:ml + 1])
                        groups = ((1, fim),)
                    else:
                        groups = ((0, fre), (1, fim))
                    for gi, grp in groups:
                        bias = b_f1re if gi == 0 else b_f1im
                        dst = g1r if gi == 0 else g1i
                        for ml in range(4):
                            mt = j * 4 + ml
                            ps_ = psp.tile([P, 512], F32, tag="mm", name="psf1")
                            run_group(ps_, grp, n2r, n2i, ml, 0)
                            nc.scalar.activation(out=dst[:, mt, :], in_=ps_,
                                                 func=AF.Gelu_apprx_tanh,
                                                 bias=bias[:, mt:mt + 1])

                # f2: single pack sweep, all 8 psum banks held
                # (LN's st/lnv psum tags are free by now)
                ps_st = pst.tile([P, 2, 512], F32, tag="st", name="pf2st")
                ps_lnv = pst.tile([P, 2, 512], F32, tag="lnv", name="pf2lnv")
                psums = [psp.tile([P, 512], F32, tag="mm", name="pf2a"),
                         psp.tile([P, 512], F32, tag="mm", name="pf2b"),
                         psa.tile([P, 512], F32, tag="av", name="pf2c"),
                         psa.tile([P, 512], F32, tag="av", name="pf2d"),
                         ps_st[:, 0, :], ps_st[:, 1, :],
                         ps_lnv[:, 0, :], ps_lnv[:, 1, :]]
                for j in range(4):
                    pk = loadpack(wf2_d[j], f"wf2p_{j}", q=j % 2)
                    f2a, f2b, f2c = msec(pk, 0), msec(pk, 1), msec(pk, 2)
                    for gi in range(2):
                        pairs = ((f2a, 0), (f2c, 1)) if gi == 0 else \
                                ((f2b, 0), (f2a, 1))
                        for mt in range(DT):
                            i = 0
                            for m_, which in pairs:
                                r_ = g1r if which == 0 else g1i
                                for kl in range(4):
                                    nc.tensor.matmul(
                                        psums[gi * 4 + mt],
                                        m_[:, kl, mt * P:(mt + 1) * P],
                                        r_[:, j * 4 + kl, :],
                                        start=(j == 0 and i == 0),
                                        stop=(j == 3 and i == 7))
                                    i += 1
                ov = out_d.ap().rearrange("c (o p) t -> c p o t", p=P)
                for gi in range(2):
                    bias = b_f2re if gi == 0 else b_f2im
                    x2s = x2r if gi == 0 else x2i
                    for mt in range(DT):
                        fg = tp.tile([P, TQ], F32, tag="og", name="fg")
                        nc.scalar.activation(out=fg, in_=psums[gi * 4 + mt],
                                             func=AF.Identity,
                                             bias=bias[:, mt:mt + 1])
                        oc_ = tp.tile([P, TQ], F32, tag="outc", bufs=2, name="oc")
                        nc.vector.tensor_tensor(out=oc_, in0=fg,
                                                in1=x2s[:, mt, :], op=ALU.add)
                        dma(ov[gi, :, mt, :], oc_)


            for _rep in range(reps):
                emit()

    _split_dma_waits(nc)
    return nc


def _split_dma_waits(nc):
    """Walrus's DIRECT2D DMA encoding takes one sync wait; move extra
    waits onto a preceding sequencer EventSemaphore on the same engine."""
    f = nc.m.functions[0]
    for blk in f.blocks:
        out = []
        for ins in blk.instructions:
            si = getattr(ins, 'sync_info', None)
            tn = type(ins).__name__
            lim = 1
            if si is not None and si.on_wait and len(si.on_wait) > lim:
                waits = list(si.on_wait)
                extra = waits[:-lim]
                si.on_wait = waits[-lim:]
                k = 0
                while extra:
                    ev = mybir.InstEventSemaphore(
                        name=f"{ins.name}_wsplit{k}",
                        engine=ins.engine,
                        ins=[], outs=[],
                        sync_info=mybir.SyncInfo(on_wait=extra[:2],
                                                 on_update=[]),
                    )
                    out.append(ev)
                    extra = extra[2:]
                    k += 1
            out.append(ins)
        blk.instructions = out


_NC_CACHE = None


def _get_nc():
    global _NC_CACHE
    if _NC_CACHE is None:
        _NC_CACHE = build_nc()
    return _NC_CACHE


TRACE = False
LAST_RESULT = None


def kernel(**inputs):
    global LAST_RESULT
    nc = _get_nc()
    in_maps = []
    for c in range(NCORES):
        in_maps.append(_prep_core(inputs, c // 2, c % 2))
    res = run_bass_kernel_spmd(nc, in_maps, list(range(NCORES)),
                               trace=TRACE)
    LAST_RESULT = res
    out = np.empty((2, B, T, D), np.float32)
    for c in range(NCORES):
        b, half = c // 2, c % 2
        y = res.results[c]["outT"]          # [2, D, TQ]
        out[:, b, half * TQ:(half + 1) * TQ, :] = y.transpose(0, 2, 1)
    return out
